# revision 3
# baseline (speedup 1.0000x reference)
"""MinLSTM Trainium2 kernel (8-core data-parallel over batch).

Math (per batch):
  preacts: F = x@Wf.T+bf, I = x@Wi.T+bi, Hp = x@Wh.T+bh      [T, H]
  sf=sigmoid(F), si=sigmoid(I)
  f_gate = sf/(sf+si), i_gate = si/(sf+si)  (normalized gates; f+i=1)
  g(z) = max(sigmoid(z), z+0.5)
  v = i_gate * g(Hp)
  h[0] = g(h_0);  h[t] = f_gate[t]*h[t-1] + v[t]   (linear recurrence)
Output: [T+1, H] per batch.

HW mapping per core (1 batch):
  - x transposed on PE (fp32 transpose) into xT [D, T] (fp32r) so matmuls
    contract d on partitions and produce [h, t] tiles.
  - W rows transposed per h-block into lhsT tiles (fp32r).
  - fp32r matmuls (1 cyc/row at N=512) accumulate preacts in PSUM.
  - Gate math: ACT sigmoids w/ fused per-partition bias from PSUM; DVE
    for normalize/max/mul; tensor_tensor_scan for the recurrence.
  - Scan output transposed back [h,t]->[t,h] on PE and DMA'd out.
"""
import sys

sys.path.insert(0, "/opt/trn_rl_repo")
import numpy as np

B, T, D, H = 8, 2048, 1024, 1024
N_CORES = 8
P = 128
TCH = 512
N_TC = T // TCH        # 4 time chunks
HB = H // P            # 8 h blocks
KD = D // P            # 8 contraction blocks
TS = T // P            # 16 time sub-tiles

_cache = {}


def _build_nc():
    import concourse.bacc as bacc
    import concourse.tile as tile
    from concourse import mybir
    from concourse.masks import make_identity

    fp32 = mybir.dt.float32
    fp32r = mybir.dt.float32r
    ACT = mybir.ActivationFunctionType
    ALU = mybir.AluOpType

    nc = bacc.Bacc("TRN2", target_bir_lowering=False, debug=False,
                   num_devices=N_CORES)

    x = nc.dram_tensor("x", [T, D], fp32, kind="ExternalInput")
    h0 = nc.dram_tensor("h0", [1, H], fp32, kind="ExternalInput")
    Wf = nc.dram_tensor("Wf", [H, D], fp32, kind="ExternalInput")
    Wi = nc.dram_tensor("Wi", [H, D], fp32, kind="ExternalInput")
    Wh = nc.dram_tensor("Wh", [H, D], fp32, kind="ExternalInput")
    bf = nc.dram_tensor("bf", [H], fp32, kind="ExternalInput")
    bi = nc.dram_tensor("bi", [H], fp32, kind="ExternalInput")
    bh = nc.dram_tensor("bh", [H], fp32, kind="ExternalInput")
    y = nc.dram_tensor("y", [T + 1, H], fp32, kind="ExternalOutput")

    Ws = [Wf, Wi, Wh]

    with tile.TileContext(nc) as tc:
        from contextlib import ExitStack
        with ExitStack() as ctx:
            consts = ctx.enter_context(tc.tile_pool(name="consts", bufs=1))
            xin_pool = ctx.enter_context(tc.tile_pool(name="xin", bufs=3))
            xt_pool = ctx.enter_context(tc.tile_pool(name="xt", bufs=1))
            win_pool = ctx.enter_context(tc.tile_pool(name="win", bufs=2))
            wt_pool = ctx.enter_context(tc.tile_pool(name="wt", bufs=2))
            gates = ctx.enter_context(tc.tile_pool(name="gates", bufs=2))
            hs_pool = ctx.enter_context(tc.tile_pool(name="hs", bufs=3))
            ost_pool = ctx.enter_context(tc.tile_pool(name="ost", bufs=3))
            mm_ps = ctx.enter_context(
                tc.tile_pool(name="mmps", bufs=2, space="PSUM"))
            tr_ps = ctx.enter_context(
                tc.tile_pool(name="trps", bufs=2, space="PSUM"))

            # ---- constants: identity, biases, h0 ----
            idn = consts.tile([P, P], fp32, name="idn")
            make_identity(nc, idn[:, :])

            def load_col(name, src_ap):
                t = consts.tile([P, HB], fp32, name=name)
                nc.sync.dma_start(
                    out=t, in_=src_ap.rearrange("(hb p) -> p hb", p=P))
                return t

            bf_t = load_col("bf_t", bf[:])
            bi_t = load_col("bi_t", bi[:])
            bh_t = load_col("bh_t", bh[:])
            h0_t = load_col("h0_t", h0[0, :])

            bhp5 = consts.tile([P, HB], fp32, name="bhp5")
            nc.vector.tensor_scalar_add(bhp5, bh_t, 0.5)
            sh0 = consts.tile([P, HB], fp32, name="sh0")
            nc.scalar.activation(sh0, h0_t, ACT.Sigmoid)
            zp0 = consts.tile([P, HB], fp32, name="zp0")
            nc.vector.tensor_scalar_add(zp0, h0_t, 0.5)
            g0 = consts.tile([P, HB], fp32, name="g0")
            nc.vector.tensor_max(g0, sh0, zp0)
            # y[0, :] = g(h_0)
            nc.sync.dma_start(
                out=y[0, :].rearrange("(hb p) -> p hb", p=P), in_=g0)

            # ---- phase 1: xT[kd] [128, T] (fp32r) via PE transpose ----
            xt = [xt_pool.tile([P, T], fp32r, name=f"xt{kd}", tag=f"xt{kd}")
                  for kd in range(KD)]
            for ts in range(TS):
                xin = xin_pool.tile([P, D], fp32, name=f"xin{ts}", tag="xin")
                nc.sync.dma_start(out=xin, in_=x[ts * P:(ts + 1) * P, :])
                for q in range(2):
                    ps = tr_ps.tile([P, TCH], fp32, name=f"xtp{ts}_{q}",
                                    tag="trps")
                    for j in range(4):
                        kd = q * 4 + j
                        nc.tensor.transpose(
                            ps[:, j * P:(j + 1) * P],
                            xin[:, kd * P:(kd + 1) * P], idn)
                    for j in range(4):
                        kd = q * 4 + j
                        eng = nc.vector if (kd % 2 == 0) else nc.scalar
                        if eng is nc.vector:
                            nc.vector.tensor_copy(
                                xt[kd][:, ts * P:(ts + 1) * P],
                                ps[:, j * P:(j + 1) * P])
                        else:
                            nc.scalar.activation(
                                xt[kd][:, ts * P:(ts + 1) * P],
                                ps[:, j * P:(j + 1) * P], ACT.Copy)

            # ---- main loop over h blocks ----
            for hb in range(HB):
                # W prep for this h block: lhsT tiles [d, h] per gate
                wt = []
                for g in range(3):
                    win = win_pool.tile([P, D], fp32, name=f"win{hb}_{g}",
                                        tag="win")
                    nc.sync.dma_start(
                        out=win, in_=Ws[g][hb * P:(hb + 1) * P, :])
                    wtg = wt_pool.tile([P, D], fp32r, name=f"wt{hb}_{g}",
                                       tag=f"wt{g}")
                    for q in range(2):
                        ps = tr_ps.tile([P, TCH], fp32,
                                        name=f"wtp{hb}_{g}_{q}", tag="trps")
                        for j in range(4):
                            kd = q * 4 + j
                            nc.tensor.transpose(
                                ps[:, j * P:(j + 1) * P],
                                win[:, kd * P:(kd + 1) * P], idn)
                        nc.vector.tensor_copy(
                            wtg[:, q * TCH:(q + 1) * TCH], ps)
                    wt.append(wtg)

                prev_hs = None
                for tc_i in range(N_TC):
                    t0 = tc_i * TCH
                    # matmuls: preacts [h=128, t=512] in PSUM
                    pre = []
                    for g in range(3):
                        psg = mm_ps.tile([P, TCH], fp32,
                                         name=f"ps{hb}_{tc_i}_{g}",
                                         tag=f"mm{g}")
                        for kd in range(KD):
                            nc.tensor.matmul(
                                psg,
                                wt[g][:, kd * P:(kd + 1) * P],
                                xt[kd][:, t0:t0 + TCH],
                                start=(kd == 0), stop=(kd == KD - 1))
                        pre.append(psg)

                    sf = gates.tile([P, TCH], fp32, name=f"sf{hb}_{tc_i}",
                                    tag="sf")
                    nc.scalar.activation(sf, pre[0], ACT.Sigmoid,
                                         bias=bf_t[:, hb:hb + 1])
                    si = gates.tile([P, TCH], fp32, name=f"si{hb}_{tc_i}",
                                    tag="si")
                    nc.scalar.activation(si, pre[1], ACT.Sigmoid,
                                         bias=bi_t[:, hb:hb + 1])
                    sh = gates.tile([P, TCH], fp32, name=f"sh{hb}_{tc_i}",
                                    tag="sh")
                    nc.scalar.activation(sh, pre[2], ACT.Sigmoid,
                                         bias=bh_t[:, hb:hb + 1])
                    zp = gates.tile([P, TCH], fp32, name=f"zp{hb}_{tc_i}",
                                    tag="zp")
                    nc.scalar.activation(zp, pre[2], ACT.Identity,
                                         bias=bhp5[:, hb:hb + 1])

                    den = gates.tile([P, TCH], fp32, name=f"den{hb}_{tc_i}",
                                     tag="den")
                    nc.vector.tensor_add(den, sf, si)
                    rec = gates.tile([P, TCH], fp32, name=f"rec{hb}_{tc_i}",
                                     tag="rec")
                    nc.vector.reciprocal(rec, den)
                    fg = gates.tile([P, TCH], fp32, name=f"fg{hb}_{tc_i}",
                                    tag="fg")
                    nc.vector.tensor_mul(fg, sf, rec)
                    ig = gates.tile([P, TCH], fp32, name=f"ig{hb}_{tc_i}",
                                    tag="ig")
                    nc.vector.tensor_scalar(out=ig, in0=fg, scalar1=-1.0,
                                            scalar2=1.0, op0=ALU.mult,
                                            op1=ALU.add)
                    gg = gates.tile([P, TCH], fp32, name=f"gg{hb}_{tc_i}",
                                    tag="gg")
                    nc.vector.tensor_max(gg, sh, zp)
                    vv = gates.tile([P, TCH], fp32, name=f"vv{hb}_{tc_i}",
                                    tag="vv")
                    nc.vector.tensor_mul(vv, ig, gg)

                    hs = hs_pool.tile([P, TCH], fp32, name=f"hs{hb}_{tc_i}",
                                      tag="hs")
                    init = (g0[:, hb:hb + 1] if tc_i == 0
                            else prev_hs[:, TCH - 1:TCH])
                    nc.vector.tensor_tensor_scan(hs, fg, vv, init,
                                                 op0=ALU.mult, op1=ALU.add)
                    prev_hs = hs

                    # transpose back to [t, h] and store
                    pso = tr_ps.tile([P, TCH], fp32, name=f"pso{hb}_{tc_i}",
                                     tag="trps")
                    for tb in range(4):
                        nc.tensor.transpose(
                            pso[:, tb * P:(tb + 1) * P],
                            hs[:, tb * P:(tb + 1) * P], idn)
                    ost = ost_pool.tile([P, TCH], fp32,
                                        name=f"ost{hb}_{tc_i}", tag="ost")
                    nc.scalar.activation(ost, pso, ACT.Copy)
                    y_ap = y[1 + t0:1 + t0 + TCH, hb * P:(hb + 1) * P]
                    nc.sync.dma_start(
                        out=y_ap.rearrange("(tb p) h -> p tb h", p=P),
                        in_=ost.rearrange("p (tb h) -> p tb h", tb=4))

    nc.compile()
    return nc


def _get_nc():
    if "nc" not in _cache:
        _cache["nc"] = _build_nc()
    return _cache["nc"]


def _run(inputs, trace=False, **kw):
    from concourse.bass_utils import run_bass_kernel_spmd

    nc = _get_nc()
    x = np.ascontiguousarray(inputs["x"], dtype=np.float32)
    h_0 = np.ascontiguousarray(inputs["h_0"], dtype=np.float32)
    shared = {
        "Wf": np.ascontiguousarray(inputs["Wf"], dtype=np.float32),
        "Wi": np.ascontiguousarray(inputs["Wi"], dtype=np.float32),
        "Wh": np.ascontiguousarray(inputs["Wh"], dtype=np.float32),
        "bf": np.ascontiguousarray(inputs["bf"], dtype=np.float32),
        "bi": np.ascontiguousarray(inputs["bi"], dtype=np.float32),
        "bh": np.ascontiguousarray(inputs["bh"], dtype=np.float32),
    }
    in_maps = []
    for b in range(B):
        m = {"x": x[b], "h0": h_0[b], **shared}
        in_maps.append(m)
    res = run_bass_kernel_spmd(nc, in_maps, list(range(N_CORES)),
                               trace=trace, **kw)
    out = np.stack([res.results[b]["y"] for b in range(B)], axis=0)
    return out, res


def kernel(**inputs) -> np.ndarray:
    out, _ = _run(inputs, trace=False)
    return out


# revision 7
# speedup vs baseline: 1.2333x; 1.2333x over previous
"""MinLSTM Trainium2 kernel (8-core data-parallel over batch).

Math (per batch):
  preacts: F = x@Wf.T+bf, I = x@Wi.T+bi, Hp = x@Wh.T+bh      [T, H]
  sf=sigmoid(F), si=sigmoid(I)
  f_gate = sf/(sf+si)  (normalized gates; f+i=1)
  g(z) = max(sigmoid(z), z+0.5)
  h[0] = g(h_0);  h[t] = f_gate[t]*h[t-1] + (1-f_gate[t])*g(Hp[t])
Output: [T+1, H] per batch.

HW mapping per core (1 batch):
  - x transposed on PE into xT tiles [d,t] (fp32r) so matmuls contract d on
    partitions and produce [h, t] preact tiles; W rows transposed per
    h-block into fp32r lhsT tiles.
  - fp32r matmuls (1 cyc/row at N=512) accumulate preacts in PSUM.
  - ACT: sigmoids with fused per-partition bias, reading PSUM.
  - DVE: g via fused (Hp+b+0.5) max sh; normalization via reciprocal;
    v' = (f-1)*g fused; recurrence via tensor_tensor_scan (op1=subtract).
  - Scan output transposed back [h,t]->[t,h] on PE (delayed one h-block to
    keep PE dense) and DMA'd out.
"""
import sys

sys.path.insert(0, "/opt/trn_rl_repo")
import numpy as np

B, T, D, H = 8, 2048, 1024, 1024
N_CORES = 8
P = 128
TCH = 512
N_TC = T // TCH        # 4 time chunks
HB = H // P            # 8 h blocks
KD = D // P            # 8 contraction blocks
TS = T // P            # 16 time sub-tiles

_cache = {}


def _build_nc():
    import concourse.bacc as bacc
    import concourse.tile as tile
    from concourse import mybir
    from concourse.masks import make_identity
    from contextlib import ExitStack

    fp32 = mybir.dt.float32
    fp32r = mybir.dt.float32r
    ACT = mybir.ActivationFunctionType
    ALU = mybir.AluOpType

    nc = bacc.Bacc("TRN2", target_bir_lowering=False, debug=False,
                   num_devices=N_CORES)

    x = nc.dram_tensor("x", [T, D], fp32, kind="ExternalInput")
    h0 = nc.dram_tensor("h0", [1, H], fp32, kind="ExternalInput")
    Wf = nc.dram_tensor("Wf", [H, D], fp32, kind="ExternalInput")
    Wi = nc.dram_tensor("Wi", [H, D], fp32, kind="ExternalInput")
    Wh = nc.dram_tensor("Wh", [H, D], fp32, kind="ExternalInput")
    bf = nc.dram_tensor("bf", [H], fp32, kind="ExternalInput")
    bi = nc.dram_tensor("bi", [H], fp32, kind="ExternalInput")
    bh = nc.dram_tensor("bh", [H], fp32, kind="ExternalInput")
    y = nc.dram_tensor("y", [T + 1, H], fp32, kind="ExternalOutput")

    Ws = [Wf, Wi, Wh]

    with tile.TileContext(nc) as tc:
        with ExitStack() as ctx:
            consts = ctx.enter_context(tc.tile_pool(name="consts", bufs=1))
            xin_pool = ctx.enter_context(tc.tile_pool(name="xin", bufs=4))
            xt_pool = ctx.enter_context(tc.tile_pool(name="xt", bufs=1))
            win_pool = ctx.enter_context(tc.tile_pool(name="win", bufs=2))
            wt_pool = ctx.enter_context(tc.tile_pool(name="wt", bufs=2))
            gates = ctx.enter_context(tc.tile_pool(name="gates", bufs=2))
            hs_pool = ctx.enter_context(tc.tile_pool(name="hs", bufs=9))
            ost_pool = ctx.enter_context(tc.tile_pool(name="ost", bufs=3))
            mm_ps = ctx.enter_context(
                tc.tile_pool(name="mmps", bufs=4, space="PSUM"))
            wx_ps = ctx.enter_context(
                tc.tile_pool(name="wxps", bufs=2, space="PSUM"))
            out_ps = ctx.enter_context(
                tc.tile_pool(name="outps", bufs=2, space="PSUM"))

            # ---- constants: identity, biases, h0 ----
            idn = consts.tile([P, P], fp32, name="idn")
            make_identity(nc, idn[:, :])

            def load_col(name, src_ap):
                t = consts.tile([P, HB], fp32, name=name)
                nc.sync.dma_start(
                    out=t, in_=src_ap.rearrange("(hb p) -> p hb", p=P))
                return t

            bf_t = load_col("bf_t", bf[:])
            bi_t = load_col("bi_t", bi[:])
            bh_t = load_col("bh_t", bh[:])
            h0_t = load_col("h0_t", h0[0, :])

            bhp5 = consts.tile([P, HB], fp32, name="bhp5")
            nc.vector.tensor_scalar_add(bhp5, bh_t, 0.5)
            sh0 = consts.tile([P, HB], fp32, name="sh0")
            nc.scalar.activation(sh0, h0_t, ACT.Sigmoid)
            g0 = consts.tile([P, HB], fp32, name="g0")
            # g0 = max(h0 + 0.5, sigmoid(h0))
            nc.vector.scalar_tensor_tensor(g0, h0_t, 0.5, sh0,
                                           op0=ALU.add, op1=ALU.max)
            nc.sync.dma_start(
                out=y[0, :].rearrange("(hb p) -> p hb", p=P), in_=g0)

            # xt[kd][tc]: [128, TCH] fp32r tiles of x^T
            xt = [[None] * N_TC for _ in range(KD)]

            def emit_xT(tc_i):
                for kd in range(KD):
                    xt[kd][tc_i] = xt_pool.tile(
                        [P, TCH], fp32r, name=f"xt{kd}_{tc_i}",
                        tag=f"xt{kd}_{tc_i}")
                for j in range(4):
                    ts = tc_i * 4 + j
                    xin = xin_pool.tile([P, D], fp32, name=f"xin{ts}",
                                        tag="xin")
                    nc.sync.dma_start(out=xin, in_=x[ts * P:(ts + 1) * P, :])
                    for q in range(2):
                        ps = wx_ps.tile([P, TCH], fp32, name=f"xtp{ts}_{q}",
                                        tag="wxps")
                        for jj in range(4):
                            kd = q * 4 + jj
                            nc.tensor.transpose(
                                ps[:, jj * P:(jj + 1) * P],
                                xin[:, kd * P:(kd + 1) * P], idn)
                        for jj in range(4):
                            kd = q * 4 + jj
                            nc.scalar.activation(
                                xt[kd][tc_i][:, j * P:(j + 1) * P],
                                ps[:, jj * P:(jj + 1) * P], ACT.Copy)

            wt_tiles = [None] * HB

            def emit_wprep(hb):
                wt = []
                for g in range(3):
                    win = win_pool.tile([P, D], fp32, name=f"win{hb}_{g}",
                                        tag="win")
                    nc.sync.dma_start(
                        out=win, in_=Ws[g][hb * P:(hb + 1) * P, :])
                    wtg = wt_pool.tile([P, D], fp32r, name=f"wt{hb}_{g}",
                                       tag=f"wt{g}")
                    for q in range(2):
                        ps = wx_ps.tile([P, TCH], fp32,
                                        name=f"wtp{hb}_{g}_{q}", tag="wxps")
                        for j in range(4):
                            kd = q * 4 + j
                            nc.tensor.transpose(
                                ps[:, j * P:(j + 1) * P],
                                win[:, kd * P:(kd + 1) * P], idn)
                        nc.scalar.activation(
                            wtg[:, q * TCH:(q + 1) * TCH], ps, ACT.Copy)
                    wt.append(wtg)
                wt_tiles[hb] = wt

            hs_tiles = [[None] * N_TC for _ in range(HB)]
            prev_hs_map = {}

            def emit_compute(hb, tcs=None):
                wt = wt_tiles[hb]
                prev_hs = prev_hs_map.get(hb)
                for tc_i in (range(N_TC) if tcs is None else tcs):
                    t0 = tc_i * TCH
                    pre = []
                    for g in range(3):
                        psg = mm_ps.tile([P, TCH], fp32,
                                         name=f"ps{hb}_{tc_i}_{g}", tag="mm")
                        for kd in range(KD):
                            nc.tensor.matmul(
                                psg,
                                wt[g][:, kd * P:(kd + 1) * P],
                                xt[kd][tc_i],
                                start=(kd == 0), stop=(kd == KD - 1))
                        pre.append(psg)

                    sf = gates.tile([P, TCH], fp32, name=f"sf{hb}_{tc_i}",
                                    tag="sf")
                    nc.scalar.activation(sf, pre[0], ACT.Sigmoid,
                                         bias=bf_t[:, hb:hb + 1])
                    si = gates.tile([P, TCH], fp32, name=f"si{hb}_{tc_i}",
                                    tag="si")
                    nc.scalar.activation(si, pre[1], ACT.Sigmoid,
                                         bias=bi_t[:, hb:hb + 1])
                    sh = gates.tile([P, TCH], fp32, name=f"sh{hb}_{tc_i}",
                                    tag="sh")
                    nc.scalar.activation(sh, pre[2], ACT.Sigmoid,
                                         bias=bh_t[:, hb:hb + 1])
                    # g = max(Hp + bh + 0.5, sigmoid(Hp + bh))
                    gg = gates.tile([P, TCH], fp32, name=f"gg{hb}_{tc_i}",
                                    tag="gg")
                    nc.vector.scalar_tensor_tensor(
                        gg, pre[2], bhp5[:, hb:hb + 1], sh,
                        op0=ALU.add, op1=ALU.max)

                    den = gates.tile([P, TCH], fp32, name=f"den{hb}_{tc_i}",
                                     tag="den")
                    nc.vector.tensor_add(den, sf, si)
                    rec = gates.tile([P, TCH], fp32, name=f"rec{hb}_{tc_i}",
                                     tag="rec")
                    nc.vector.reciprocal(rec, den)
                    fg = gates.tile([P, TCH], fp32, name=f"fg{hb}_{tc_i}",
                                    tag="fg")
                    nc.vector.tensor_mul(fg, sf, rec)
                    # nv = (f - 1) * g  (so scan's op1=subtract adds (1-f)*g)
                    nv = gates.tile([P, TCH], fp32, name=f"nv{hb}_{tc_i}",
                                    tag="nv")
                    nc.vector.scalar_tensor_tensor(nv, fg, 1.0, gg,
                                                   op0=ALU.subtract,
                                                   op1=ALU.mult)

                    hs = hs_pool.tile([P, TCH], fp32, name=f"hs{hb}_{tc_i}",
                                      tag="hs")
                    init = (g0[:, hb:hb + 1] if tc_i == 0
                            else prev_hs[:, TCH - 1:TCH])
                    nc.vector.tensor_tensor_scan(hs, fg, nv, init,
                                                 op0=ALU.mult,
                                                 op1=ALU.subtract)
                    prev_hs = hs
                    hs_tiles[hb][tc_i] = hs
                prev_hs_map[hb] = prev_hs

            def emit_out(hb):
                for tc_i in range(N_TC):
                    t0 = tc_i * TCH
                    hs = hs_tiles[hb][tc_i]
                    pso = out_ps.tile([P, TCH], fp32,
                                      name=f"pso{hb}_{tc_i}", tag="outps")
                    for tb in range(4):
                        nc.tensor.transpose(
                            pso[:, tb * P:(tb + 1) * P],
                            hs[:, tb * P:(tb + 1) * P], idn)
                    ost = ost_pool.tile([P, TCH], fp32,
                                        name=f"ost{hb}_{tc_i}", tag="ost")
                    nc.scalar.activation(ost, pso, ACT.Copy)
                    y_ap = y[1 + t0:1 + t0 + TCH, hb * P:(hb + 1) * P]
                    nc.sync.dma_start(
                        out=y_ap.rearrange("(tb p) h -> p tb h", p=P),
                        in_=ost.rearrange("p (tb h) -> p tb h", tb=4))

            # ---- emission schedule (software-pipelined for PE density) ----
            emit_wprep(0)
            for tc_i in range(N_TC):
                emit_xT(tc_i)
                emit_compute(0, tcs=[tc_i])
            for hb in range(1, HB):
                emit_wprep(hb)
                emit_compute(hb)
                emit_out(hb - 1)
            emit_out(HB - 1)

    nc.compile()
    return nc


def _get_nc():
    if "nc" not in _cache:
        _cache["nc"] = _build_nc()
    return _cache["nc"]


def _run(inputs, trace=False, **kw):
    from concourse.bass_utils import run_bass_kernel_spmd

    nc = _get_nc()
    x = np.ascontiguousarray(inputs["x"], dtype=np.float32)
    h_0 = np.ascontiguousarray(inputs["h_0"], dtype=np.float32)
    shared = {
        "Wf": np.ascontiguousarray(inputs["Wf"], dtype=np.float32),
        "Wi": np.ascontiguousarray(inputs["Wi"], dtype=np.float32),
        "Wh": np.ascontiguousarray(inputs["Wh"], dtype=np.float32),
        "bf": np.ascontiguousarray(inputs["bf"], dtype=np.float32),
        "bi": np.ascontiguousarray(inputs["bi"], dtype=np.float32),
        "bh": np.ascontiguousarray(inputs["bh"], dtype=np.float32),
    }
    in_maps = []
    for b in range(B):
        m = {"x": x[b], "h0": h_0[b], **shared}
        in_maps.append(m)
    res = run_bass_kernel_spmd(nc, in_maps, list(range(N_CORES)),
                               trace=trace, **kw)
    out = np.stack([res.results[b]["y"] for b in range(B)], axis=0)
    return out, res


def kernel(**inputs) -> np.ndarray:
    out, _ = _run(inputs, trace=False)
    return out
